# revision 27
# baseline (speedup 1.0000x reference)
"""Trainium2 Bass kernel for nn_BraidCrossing (B=8, T=2048, D=2048, NG=3).

Math notes
----------
reference computes:
    pair  = [x_t, x_{t+1}]                       (B, T-1, 2D)
    h     = gelu(pair @ W1.T + b1)
    logit = h @ W2.T + b2                        (B, T-1, 2*NG)
    scale = mean(softmax(logit, -1), -1)         == 1/(2*NG) EXACTLY (mean of a
                                                 softmax over the same axis)
    P     = x @ Wp.T + bp
    tmp_t = LN(x_t + P_{t-1} * scale)   t>=1 ;  tmp_0 = x_0
    out_t = LN(tmp_t + P_{t+1} * scale) t<=T-2; out_{T-1} = tmp_{T-1}

Because scale is a constant (1/(2*NG); setup has bp=0, gamma=1, beta=0), the
entire W1/W2/gelu branch is dead code.  The device kernel computes
Q = x @ Wp.T, then the two chained layernorms (scale folded into the adds).

Key structural tricks:
 * LN *means* are linear in x -> computed on the HOST exactly:
   mu1[t] = (sum_e x[t] + scale * Qsum[t-1]) / D, Qsum[t] = x[t] . rowsum(WpT),
   and mean(LN1_out) == 0 by construction so mu2[t] = scale * Qsum[t+1] / D.
   Only the variances need a device-side quadratic reduction, done with
   ACT activation(Square, accum_out), one pass per LN.  (NOTE:
   vector.tensor_tensor_reduce crashes the exec unit on this runtime —
   NRT_EXEC_UNIT_UNRECOVERABLE — do not use it.)
 * UNIFORM 126-row tiles: tile i handles out rows t = i*128+1 .. i*128+126.
   The 2 rows at each tile boundary (t = i*128+127, i*128+128), plus t=0 and
   t=T-1, are single matvecs -> computed on host (32 rows/core, one small
   batched GEMM).  This removes ALL cross-tile dependencies (the shifted-Q
   operand q[t+1] comes from rows 2..127 of the SAME tile's Q) and all tiny
   2-row DMAs (whose completion semaphores measured ~8-10us latency when the
   HWDGE ring idles — that stall serialized the old drain).
 * PSUM evacuation of tile i happens one pipeline step AFTER its matmuls,
   as a raw tensor_copy (scale folded into the downstream STT adds), so no
   engine dependency ever gates the PE matmul stream (HAM stays warm).
 * Output stored bf16, upcast on host (halves store traffic).
 * Matmul loop kp-outer/n-inner: 4 consecutive MMs share one LDWEIGHTS
   content; n-outer ordering makes LDWEIGHTS rate-limiting (216->259ns/MM).

Precision: GEMM in fp8 e4m3 (DoubleRow, K=256/matmul, fp32 PSUM); LN chain in
bf16 with fp32 statistics.  Measured max rel err ~1.3e-2 (gate 2e-2).

Sharding: data-parallel over batch, one batch per NeuronCore (8 cores).
"""
import numpy as np
import ml_dtypes

import concourse.bass as bass
from concourse import bacc
import concourse.mybir as mybir
import concourse.tile as tile
from concourse.bass_utils import run_bass_kernel_spmd

FP32 = mybir.dt.float32
BF16 = mybir.dt.bfloat16
F8 = mybir.dt.float8e4
AF = mybir.ActivationFunctionType
ALU = mybir.AluOpType
DR = mybir.MatmulPerfMode.DoubleRow

B, T, D = 8, 2048, 2048
P = 128                # partitions
NR = P - 2             # 126 device rows per tile
NT = T // P            # 16 t-tiles
NK = D // P            # 16 contraction k-tiles
NKP = NK // 2          # 8 k-pairs (DoubleRow: 256 contraction per matmul)
NE = D // 512          # 4 psum-bank chunks along e
EPS = 1e-5
N_CORES = 8

SX = 16.0              # fp8 pre-scale for x
SW = 1024.0            # fp8 pre-scale for Wp.T
F8NP = ml_dtypes.float8_e4m3
BF16NP = ml_dtypes.bfloat16

_cache = {}


def _build(scale: float):
    # PSUM = (x*SX) @ (WpT*SW); q_raw = copy(PSUM);
    # q = q_raw * qscale applied inside the two STT adds
    qscale = float(scale) / (SX * SW)

    nc = bacc.Bacc("TRN2", target_bir_lowering=False, debug=False)
    xb_d = nc.declare_dram_parameter("xb", [T, D], BF16, isOutput=False)
    # host-tiled transpose: xTt[i, p, k, tt] = x[i*128+tt, k*128+p] (fp8),
    # so lhsT slice [:, 2kp:2kp+2, :] is the DoubleRow stationary operand
    xTt_d = nc.declare_dram_parameter("xTt", [NT, P, NK, P], F8, isOutput=False)
    # w8[kp, p, s, e] = WpT[(2kp+s)*128+p, e] * SW (fp8)
    w8_d = nc.declare_dram_parameter("w8", [NKP, P, 2, D], F8, isOutput=False)
    # host LN stats: columns [mu1 | c1 | mu2 | c2], c = (eps - mu^2) * D
    st_d = nc.declare_dram_parameter("st", [P, 4 * NT], FP32, isOutput=False)
    out_d = nc.declare_dram_parameter("out", [T, D], BF16, isOutput=True)

    xb_ap = xb_d.ap()
    out_ap = out_d.ap()
    xTt_ap = xTt_d.ap()

    with tile.TileContext(nc) as tc:
        with tc.tile_pool(name="wp", bufs=1) as wp_pool, \
             tc.tile_pool(name="xt", bufs=3) as xt_pool, \
             tc.tile_pool(name="q", bufs=3) as q_pool, \
             tc.tile_pool(name="xv", bufs=4) as xv_pool, \
             tc.tile_pool(name="qs", bufs=3) as qs_pool, \
             tc.tile_pool(name="v1", bufs=3) as v1_pool, \
             tc.tile_pool(name="sq", bufs=2) as sq_pool, \
             tc.tile_pool(name="v2", bufs=3) as v2_pool, \
             tc.tile_pool(name="o", bufs=3) as o_pool, \
             tc.tile_pool(name="stat", bufs=4) as stat_pool, \
             tc.tile_pool(name="ps", bufs=2, space="PSUM") as ps_pool:

            st_sb = stat_pool.tile([P, 4 * NT], FP32, tag="st", bufs=1)

            def mu1(i):
                return st_sb[:, i:i + 1]

            def c1(i):
                return st_sb[:, NT + i:NT + i + 1]

            def mu2(i):
                return st_sb[:, 2 * NT + i:2 * NT + i + 1]

            def c2(i):
                return st_sb[:, 3 * NT + i:3 * NT + i + 1]

            # prefetch first lhsT tile and x rows, then stream the fp8
            # weights kp-ordered across both HWDGE rings: front(0)'s kp=0
            # matmuls start as soon as xt(0) and wp[0] land
            xt_pre = {}
            xt0 = xt_pool.tile([P, NK, P], F8, tag="xt")
            nc.sync.dma_start(out=xt0, in_=xTt_ap[0])
            xt_pre[0] = xt0
            wp = []
            for kp in range(NKP):
                w = wp_pool.tile([P, 2, D], F8, tag=f"wp{kp}", bufs=1)
                wp.append(w)
            nc.scalar.dma_start(out=wp[0], in_=w8_d.ap()[0])
            nc.sync.dma_start(out=wp[1], in_=w8_d.ap()[1])
            xt1 = xt_pool.tile([P, NK, P], F8, tag="xt")
            nc.sync.dma_start(out=xt1, in_=xTt_ap[1])
            xt_pre[1] = xt1
            for kp in range(2, NKP):
                eng = nc.scalar if kp % 2 == 0 else nc.sync
                eng.dma_start(out=wp[kp], in_=w8_d.ap()[kp])
            xv_pre = {}
            xv0 = xv_pool.tile([P, D], BF16, tag="xv")
            nc.sync.dma_start(out=xv0[:NR, :], in_=xb_ap[1:1 + NR, :])
            xv_pre[0] = xv0
            nc.scalar.dma_start(out=st_sb, in_=st_d.ap())

            qp_of = {}
            q_of = {}
            qs_of = {}
            v1_of = {}
            v2_of = {}
            rs1_of = {}

            def front(i):
                xt_i = xt_pre.pop(i)
                qp = ps_pool.tile([P, D], FP32, tag="qps", bufs=2)
                for kp in range(NKP):
                    lhsT = xt_i[:, 2 * kp:2 * kp + 2, :]
                    for n in range(NE):
                        nc.tensor.matmul(qp[:, n * 512:(n + 1) * 512],
                                         lhsT,
                                         wp[kp][:, :, n * 512:(n + 1) * 512],
                                         start=(kp == 0), stop=(kp == NKP - 1),
                                         perf_mode=DR)
                qp_of[i] = qp
                if i + 2 < NT:
                    xt_n = xt_pool.tile([P, NK, P], F8, tag="xt")
                    nc.sync.dma_start(out=xt_n, in_=xTt_ap[i + 2])
                    xt_pre[i + 2] = xt_n
                if i + 1 < NT:
                    xv_n = xv_pool.tile([P, D], BF16, tag="xv")
                    t0 = (i + 1) * P + 1
                    nc.sync.dma_start(out=xv_n[:NR, :], in_=xb_ap[t0:t0 + NR, :])
                    xv_pre[i + 1] = xv_n

            def evac(i, chunks=1):
                # PSUM -> SBUF raw copy (bf16) on DVE, one step after front(i)
                q_i = q_pool.tile([P, D], BF16, tag="q")
                qp = qp_of.pop(i)
                cw = D // chunks
                for c in range(chunks):
                    nc.vector.tensor_copy(out=q_i[:, c * cw:(c + 1) * cw],
                                          in_=qp[:, c * cw:(c + 1) * cw])
                q_of[i] = q_i

            def qs_copy(i):
                # shifted-Q operand for the second LN: qs[j] = Q_raw[i*128+2+j]
                # single 126-row HWDGE copy from THIS tile's q only
                qs_i = qs_pool.tile([P, D], BF16, tag="qs")
                nc.sync.dma_start(out=qs_i[0:NR, :], in_=q_of.pop(i)[2:P, :])
                qs_of[i] = qs_i

            sq1_st = {}
            sq2_st = {}

            def add1(i):
                # v1 = x'_t + Q_raw[t-1]  (primed units, plain add)
                xv_i = xv_pre.pop(i)
                q_i = q_of[i]
                v1 = v1_pool.tile([P, D], BF16, tag="v1")
                nc.vector.tensor_add(out=v1[:NR], in0=q_i[:NR],
                                     in1=xv_i[:NR])
                v1_of[i] = v1

            def sq1_start(i):
                # S1 = sum(v1^2) via ACT Square+accum
                sq = sq_pool.tile([P, D], BF16, tag="sq")
                s1a = stat_pool.tile([P, 1], FP32, tag="s1a")
                nc.scalar.activation(out=sq[:NR], in_=v1_of[i][:NR],
                                     func=AF.Square, accum_out=s1a[:NR])
                sq1_st[i] = s1a

            def a_stats(i):
                # rs1 = 1/sigma1 (primed apply keeps LN1 output in primed units)
                s1a = sq1_st.pop(i)
                u1 = stat_pool.tile([P, 1], FP32, tag="u1")
                nc.vector.tensor_add(out=u1[:NR], in0=s1a[:NR], in1=c1(i)[:NR])
                s1 = stat_pool.tile([P, 1], FP32, tag="s1")
                nc.scalar.activation(out=s1[:NR], in_=u1[:NR], func=AF.Sqrt,
                                     scale=qscale * qscale / D)
                rs1 = stat_pool.tile([P, 1], FP32, tag="rs1")
                nc.vector.reciprocal(out=rs1[:NR], in_=s1[:NR])
                rs1_of[i] = rs1

            def apply_b(i):
                # v2 = LN1(v1)/qscale + Q_raw[t+1]
                v1 = v1_of.pop(i)
                rs1 = rs1_of.pop(i)
                v2 = v2_pool.tile([P, D], BF16, tag="v2")
                nc.vector.tensor_scalar(out=v2[:NR], in0=v1[:NR],
                                        scalar1=mu1(i)[:NR], scalar2=rs1[:NR],
                                        op0=ALU.subtract, op1=ALU.mult)
                nc.vector.tensor_add(out=v2[:NR], in0=qs_of.pop(i)[:NR],
                                     in1=v2[:NR])
                v2_of[i] = v2

            def sq2_start(i):
                sqf = sq_pool.tile([P, D], BF16, tag="sq2")
                s2a = stat_pool.tile([P, 1], FP32, tag="s2a")
                nc.scalar.activation(out=sqf[:NR], in_=v2_of[i][:NR],
                                     func=AF.Square, accum_out=s2a[:NR])
                sq2_st[i] = s2a

            def h2_rest(i):
                v2 = v2_of.pop(i)
                s2a = sq2_st.pop(i)
                u2 = stat_pool.tile([P, 1], FP32, tag="u2")
                nc.vector.tensor_add(out=u2[:NR], in0=s2a[:NR], in1=c2(i)[:NR])
                s2 = stat_pool.tile([P, 1], FP32, tag="s2")
                nc.scalar.activation(out=s2[:NR], in_=u2[:NR], func=AF.Sqrt,
                                     scale=qscale * qscale / D)
                rs2 = stat_pool.tile([P, 1], FP32, tag="rs2")
                nc.vector.reciprocal(out=rs2[:NR], in_=s2[:NR])
                # rs2' = qscale/sigma2: converts primed v2 back to unit scale
                rs2p = stat_pool.tile([P, 1], FP32, tag="rs2p")
                nc.vector.tensor_scalar_mul(rs2p[:NR], rs2[:NR], qscale)
                o = o_pool.tile([P, D], BF16, tag="o")
                nc.vector.tensor_scalar(out=o[:NR], in0=v2[:NR],
                                        scalar1=mu2(i)[:NR], scalar2=rs2p[:NR],
                                        op0=ALU.subtract, op1=ALU.mult)
                t0 = i * P + 1
                nc.sync.dma_start(out=out_ap[t0:t0 + NR, :], in_=o[:NR])

            # 4-stage software pipeline with hand-ordered per-engine FIFOs:
            # front(i) | evac+add1+stats(i)@+1 | apply_b(i)@+2 | half2(i)@+3.
            # Emission order is chosen so no engine queue head ever waits
            # long on another engine's in-flight op (the strict FIFOs would
            # otherwise serialize: an op's wait blocks everything behind it).
            def step(i):
                front(i)
                if i >= 1:
                    evac(i - 1)           # DVE: frees PSUM early
                if i >= 3:
                    sq2_start(i - 3)      # ACT: leads the ACT queue, dep ready
                if i >= 1:
                    add1(i - 1)           # DVE
                    sq1_start(i - 1)      # ACT: after Sq2 in queue
                    qs_copy(i - 1)        # sync
                if i >= 2:
                    apply_b(i - 2)        # DVE
                if i >= 3:
                    h2_rest(i - 3)        # DVE smalls + ACT sqrt2 + store
                if i >= 1:
                    a_stats(i - 1)        # DVE smalls + ACT sqrt1

            def front_last(i):
                # n-outer: each 512-col PSUM chunk completes after its 8 kp
                # matmuls, so the drain chain starts ~3/4 of a front early.
                # (LDWEIGHTS becomes rate-limiting, +~1.4us on the stream,
                # but the chain starts ~5.5us earlier: net win on the tail.)
                xt_i = xt_pre.pop(i)
                qp = ps_pool.tile([P, D], FP32, tag="qps", bufs=2)
                for n in range(NE):
                    for kp in range(NKP):
                        lhsT = xt_i[:, 2 * kp:2 * kp + 2, :]
                        nc.tensor.matmul(qp[:, n * 512:(n + 1) * 512],
                                         lhsT,
                                         wp[kp][:, :, n * 512:(n + 1) * 512],
                                         start=(kp == 0), stop=(kp == NKP - 1),
                                         perf_mode=DR)
                qp_of[i] = qp

            def chunk_chain_last(i):
                # evac/add1/Sq1/qs of the last tile, chunked per 512-col
                # PSUM chunk so each starts as soon as its matmuls finish
                q_i = q_pool.tile([P, D], BF16, tag="q")
                qs_i = qs_pool.tile([P, D], BF16, tag="qs")
                v1 = v1_pool.tile([P, D], BF16, tag="v1")
                sq = sq_pool.tile([P, D], BF16, tag="sq")
                xv_i = xv_pre.pop(i)
                qp = qp_of.pop(i)
                accs = []
                for n in range(NE):
                    sl = slice(n * 512, (n + 1) * 512)
                    nc.vector.tensor_copy(out=q_i[:, sl], in_=qp[:, sl])
                    nc.vector.tensor_add(out=v1[:NR, sl], in0=q_i[:NR, sl],
                                         in1=xv_i[:NR, sl])
                    s1n = stat_pool.tile([P, 1], FP32, tag=f"s1c{n}")
                    nc.scalar.activation(out=sq[:NR, sl], in_=v1[:NR, sl],
                                         func=AF.Square, accum_out=s1n[:NR])
                    nc.sync.dma_start(out=qs_i[0:NR, sl], in_=q_i[2:P, sl])
                    accs.append(s1n)
                ua = stat_pool.tile([P, 1], FP32, tag="u1a")
                nc.vector.tensor_add(out=ua[:NR], in0=accs[0][:NR],
                                     in1=accs[1][:NR])
                ub = stat_pool.tile([P, 1], FP32, tag="u1b")
                nc.vector.tensor_add(out=ub[:NR], in0=accs[2][:NR],
                                     in1=accs[3][:NR])
                nc.vector.tensor_add(out=ua[:NR], in0=ua[:NR], in1=ub[:NR])
                u1 = stat_pool.tile([P, 1], FP32, tag="u1")
                nc.vector.tensor_add(out=u1[:NR], in0=ua[:NR], in1=c1(i)[:NR])
                s1 = stat_pool.tile([P, 1], FP32, tag="s1")
                nc.scalar.activation(out=s1[:NR], in_=u1[:NR], func=AF.Sqrt,
                                     scale=qscale * qscale / D)
                rs1 = stat_pool.tile([P, 1], FP32, tag="rs1")
                nc.vector.reciprocal(out=rs1[:NR], in_=s1[:NR])
                v1_of[i] = v1
                rs1_of[i] = rs1
                qs_of[i] = qs_i

            for i in range(NT - 1):
                step(i)
            # last front + drain: tile chains are independent; emission in
            # estimated ready-time order so no engine FIFO head-blocks
            L = NT - 1
            front_last(L)
            evac(L - 1)
            sq2_start(L - 3)
            add1(L - 1)
            sq1_start(L - 1)
            qs_copy(L - 1)
            chunk_chain_last(L)
            apply_b(L - 2)
            h2_rest(L - 3)
            a_stats(L - 1)
            sq2_start(L - 2)
            apply_b(L - 1)
            h2_rest(L - 2)
            sq2_start(L - 1)
            apply_b(L)
            h2_rest(L - 1)
            sq2_start(L)
            h2_rest(L)

    nc.compile()
    return nc


def _get_program(scale: float):
    key = round(float(scale), 9)
    if key not in _cache:
        _cache[key] = _build(float(scale))
    return _cache[key]


def _identity_ln_params(bp, gamma, beta):
    return (not np.any(bp)) and (not np.any(beta)) and np.all(gamma == 1.0)


def _ln_np(v):
    mu = v.mean(-1, keepdims=True)
    var = ((v - mu) ** 2).mean(-1, keepdims=True)
    return (v - mu) / np.sqrt(var + EPS)


def _reference_numpy(x, W1, b1, W2, b2, Wp, bp, gamma, beta):
    """Exact numpy port of the jax reference (emergency fallback only)."""
    import math

    def ln(v):
        mu = v.mean(-1, keepdims=True)
        var = ((v - mu) ** 2).mean(-1, keepdims=True)
        return (v - mu) / np.sqrt(var + EPS) * gamma + beta

    erf = np.vectorize(math.erf)
    x64 = x.astype(np.float32)
    pair = np.concatenate([x64[:, :-1], x64[:, 1:]], axis=-1)
    h0 = pair @ W1.T + b1
    h = 0.5 * h0 * (1.0 + erf(h0 / np.sqrt(2.0)))
    logits = h @ W2.T + b2
    e = np.exp(logits - logits.max(-1, keepdims=True))
    sm = e / e.sum(-1, keepdims=True)
    scale = sm.mean(-1, keepdims=True)
    Pm = x64 @ Wp.T + bp
    m = Pm[:, 1:] * scale
    mp = Pm[:, :-1] * scale
    tmp = np.concatenate([x64[:, :1], ln(x64[:, 1:] + mp)], axis=1)
    out = np.concatenate([ln(tmp[:, :-1] + m), tmp[:, -1:]], axis=1)
    return out.astype(np.float32)


# device-skipped rows: t=0, T-1 and the 2 rows at each 126-row tile boundary
HOST_TS = sorted({0, T - 1} |
                 {i * P + 127 for i in range(NT - 1)} |
                 {i * P + 128 for i in range(NT - 1)})
_P_ROWS = sorted({1, T - 2} |
                 {i * P + o for i in range(NT - 1) for o in (126, 127, 128, 129)})
_P_IDX = {r: k for k, r in enumerate(_P_ROWS)}


def _stats_table(x_c, rowsum, scale, qscale):
    """Host LN stats in PRIMED units (v' = v/qscale): [P, 4*NT] fp32."""
    xsum = x_c.sum(-1, dtype=np.float64)
    Qsum = (x_c @ rowsum).astype(np.float64)
    mu1 = np.zeros(T)
    mu1[1:] = (xsum[1:] + scale * Qsum[:-1]) / D
    mu2 = np.zeros(T)
    mu2[:T - 1] = scale * Qsum[1:] / D
    t_idx = np.arange(NT)[None, :] * P + 1 + np.arange(P)[:, None]  # [P, NT]
    ok = np.arange(P)[:, None] < NR
    ti = np.minimum(t_idx, T - 1)
    m1 = np.where(ok, mu1[ti], 0.0)
    m2 = np.where(ok, mu2[ti], 0.0)
    cc1 = (EPS - m1 ** 2) * D
    cc2 = (EPS - m2 ** 2) * D
    q2 = qscale * qscale
    return np.ascontiguousarray(
        np.concatenate([m1 / qscale, cc1 / q2, m2 / qscale, cc2 / q2],
                       axis=1).astype(np.float32))


def _host_rows(x, wT, scale, out):
    """Fill the device-skipped rows exactly on host (one batched GEMM)."""
    Pn = x[:, _P_ROWS, :].astype(np.float64) @ wT.astype(np.float64)  # (B,R,D)
    for c in range(N_CORES):
        for t in HOST_TS:
            if t == 0:
                tmp = x[c, 0].astype(np.float64)
            else:
                tmp = _ln_np(x[c, t].astype(np.float64)
                             + scale * Pn[c, _P_IDX[t - 1]])
            if t == T - 1:
                out[c, t] = tmp
            else:
                out[c, t] = _ln_np(tmp + scale * Pn[c, _P_IDX[t + 1]])


def run_device(x, wT, scale, trace=False):
    """x: (B,T,D) fp32, wT: (D,D) fp32 (= Wp.T contiguous)."""
    nc = _get_program(scale)
    x8 = np.clip(x * SX, -240.0, 240.0).astype(F8NP)         # (B,T,D) fp8
    w8 = np.ascontiguousarray(
        np.clip(wT * SW, -240.0, 240.0).astype(F8NP)
        .reshape(NKP, 2, P, D).transpose(0, 2, 1, 3))        # (8,128,2,2048)
    rowsum = wT.sum(1).astype(np.float32)
    qscale = float(scale) / (SX * SW)
    in_maps = []
    for c in range(N_CORES):
        # x in primed units (x/qscale) so the device adds need no scaling
        xb = np.ascontiguousarray((x[c] * (1.0 / qscale)).astype(BF16NP))
        # xTt[i, p, k, tt] = x8[i*128+tt, k*128+p]
        xTb = np.ascontiguousarray(
            x8[c].reshape(NT, P, NK, P).transpose(0, 3, 2, 1))
        st = _stats_table(x[c], rowsum, scale, qscale)
        in_maps.append({"xb": xb, "xTt": xTb, "w8": w8, "st": st})
    res = run_bass_kernel_spmd(nc, in_maps, list(range(N_CORES)), trace=trace)
    out = np.empty((B, T, D), np.float32)
    for c in range(N_CORES):
        out[c] = res.results[c]["out"].astype(np.float32)
    _host_rows(x, wT, scale, out)
    return out, res


def kernel(x, W1, b1, W2, b2, Wp, bp, gamma, beta):
    x = np.asarray(x, dtype=np.float32)
    Wp = np.asarray(Wp, dtype=np.float32)
    bp = np.asarray(bp); gamma = np.asarray(gamma); beta = np.asarray(beta)
    b2 = np.asarray(b2)
    if x.shape != (B, T, D) or not _identity_ln_params(bp, gamma, beta):
        return _reference_numpy(np.asarray(x), np.asarray(W1), np.asarray(b1),
                                np.asarray(W2), b2, Wp, bp, gamma, beta)
    scale = 1.0 / float(b2.shape[0])
    wT = np.ascontiguousarray(Wp.T)
    out, _ = run_device(x, wT, scale, trace=False)
    return out


# revision 28
# speedup vs baseline: 1.0012x; 1.0012x over previous
"""Trainium2 Bass kernel for nn_BraidCrossing (B=8, T=2048, D=2048, NG=3).

Math notes
----------
reference computes:
    pair  = [x_t, x_{t+1}]                       (B, T-1, 2D)
    h     = gelu(pair @ W1.T + b1)
    logit = h @ W2.T + b2                        (B, T-1, 2*NG)
    scale = mean(softmax(logit, -1), -1)         == 1/(2*NG) EXACTLY (mean of a
                                                 softmax over the same axis)
    P     = x @ Wp.T + bp
    tmp_t = LN(x_t + P_{t-1} * scale)   t>=1 ;  tmp_0 = x_0
    out_t = LN(tmp_t + P_{t+1} * scale) t<=T-2; out_{T-1} = tmp_{T-1}

Because scale is a constant (1/(2*NG); setup has bp=0, gamma=1, beta=0), the
entire W1/W2/gelu branch is dead code.  The device kernel computes
Q = x @ Wp.T, then the two chained layernorms (scale folded into the adds).

Key structural tricks:
 * LN *means* are linear in x -> computed on the HOST exactly:
   mu1[t] = (sum_e x[t] + scale * Qsum[t-1]) / D, Qsum[t] = x[t] . rowsum(WpT),
   and mean(LN1_out) == 0 by construction so mu2[t] = scale * Qsum[t+1] / D.
   Only the variances need a device-side quadratic reduction, done with
   ACT activation(Square, accum_out), one pass per LN.  (NOTE:
   vector.tensor_tensor_reduce crashes the exec unit on this runtime —
   NRT_EXEC_UNIT_UNRECOVERABLE — do not use it.)
 * UNIFORM 126-row tiles: tile i handles out rows t = i*128+1 .. i*128+126.
   The 2 rows at each tile boundary (t = i*128+127, i*128+128), plus t=0 and
   t=T-1, are single matvecs -> computed on host (32 rows/core, one small
   batched GEMM).  This removes ALL cross-tile dependencies (the shifted-Q
   operand q[t+1] comes from rows 2..127 of the SAME tile's Q) and all tiny
   2-row DMAs (whose completion semaphores measured ~8-10us latency when the
   HWDGE ring idles — that stall serialized the old drain).
 * PSUM evacuation of tile i happens one pipeline step AFTER its matmuls,
   as a raw tensor_copy (scale folded into the downstream STT adds), so no
   engine dependency ever gates the PE matmul stream (HAM stays warm).
 * Output stored bf16, upcast on host (halves store traffic).
 * Matmul loop kp-outer/n-inner: 4 consecutive MMs share one LDWEIGHTS
   content; n-outer ordering makes LDWEIGHTS rate-limiting (216->259ns/MM).

Precision: GEMM in fp8 e4m3 (DoubleRow, K=256/matmul, fp32 PSUM); LN chain in
bf16 with fp32 statistics.  Measured max rel err ~1.3e-2 (gate 2e-2).

Sharding: data-parallel over batch, one batch per NeuronCore (8 cores).
"""
import numpy as np
import ml_dtypes

import concourse.bass as bass
from concourse import bacc
import concourse.mybir as mybir
import concourse.tile as tile
from concourse.bass_utils import run_bass_kernel_spmd

FP32 = mybir.dt.float32
BF16 = mybir.dt.bfloat16
F8 = mybir.dt.float8e4
AF = mybir.ActivationFunctionType
ALU = mybir.AluOpType
DR = mybir.MatmulPerfMode.DoubleRow

B, T, D = 8, 2048, 2048
P = 128                # partitions
NR = P - 2             # 126 device rows per tile
NT = T // P            # 16 t-tiles
NK = D // P            # 16 contraction k-tiles
NKP = NK // 2          # 8 k-pairs (DoubleRow: 256 contraction per matmul)
NE = D // 512          # 4 psum-bank chunks along e
EPS = 1e-5
N_CORES = 8

SX = 16.0              # fp8 pre-scale for x
SW = 1024.0            # fp8 pre-scale for Wp.T
F8NP = ml_dtypes.float8_e4m3
BF16NP = ml_dtypes.bfloat16

_cache = {}


def _build(scale: float):
    # PSUM = (x*SX) @ (WpT*SW); q_raw = copy(PSUM);
    # q = q_raw * qscale applied inside the two STT adds
    qscale = float(scale) / (SX * SW)

    nc = bacc.Bacc("TRN2", target_bir_lowering=False, debug=False)
    xb_d = nc.declare_dram_parameter("xb", [T, D], BF16, isOutput=False)
    # host-tiled transpose: xTt[i, p, k, tt] = x[i*128+tt, k*128+p] (fp8),
    # so lhsT slice [:, 2kp:2kp+2, :] is the DoubleRow stationary operand
    xTt_d = nc.declare_dram_parameter("xTt", [NT, P, NK, P], F8, isOutput=False)
    # w8[kp, p, s, e] = WpT[(2kp+s)*128+p, e] * SW (fp8)
    w8_d = nc.declare_dram_parameter("w8", [NKP, P, 2, D], F8, isOutput=False)
    # host LN stats: columns [mu1 | c1 | mu2 | c2], c = (eps - mu^2) * D
    st_d = nc.declare_dram_parameter("st", [P, 4 * NT], FP32, isOutput=False)
    out_d = nc.declare_dram_parameter("out", [T, D], BF16, isOutput=True)

    xb_ap = xb_d.ap()
    out_ap = out_d.ap()
    xTt_ap = xTt_d.ap()

    with tile.TileContext(nc) as tc:
        with tc.tile_pool(name="wp", bufs=1) as wp_pool, \
             tc.tile_pool(name="xt", bufs=3) as xt_pool, \
             tc.tile_pool(name="q", bufs=3) as q_pool, \
             tc.tile_pool(name="xv", bufs=4) as xv_pool, \
             tc.tile_pool(name="qs", bufs=3) as qs_pool, \
             tc.tile_pool(name="v1", bufs=3) as v1_pool, \
             tc.tile_pool(name="sq", bufs=2) as sq_pool, \
             tc.tile_pool(name="v2", bufs=3) as v2_pool, \
             tc.tile_pool(name="o", bufs=3) as o_pool, \
             tc.tile_pool(name="stat", bufs=4) as stat_pool, \
             tc.tile_pool(name="ps", bufs=2, space="PSUM") as ps_pool:

            st_sb = stat_pool.tile([P, 4 * NT], FP32, tag="st", bufs=1)

            def mu1(i):
                return st_sb[:, i:i + 1]

            def c1(i):
                return st_sb[:, NT + i:NT + i + 1]

            def mu2(i):
                return st_sb[:, 2 * NT + i:2 * NT + i + 1]

            def c2(i):
                return st_sb[:, 3 * NT + i:3 * NT + i + 1]

            # prefetch first lhsT tile and x rows, then stream the fp8
            # weights kp-ordered across both HWDGE rings: front(0)'s kp=0
            # matmuls start as soon as xt(0) and wp[0] land
            xt_pre = {}
            xt0 = xt_pool.tile([P, NK, P], F8, tag="xt")
            nc.sync.dma_start(out=xt0, in_=xTt_ap[0])
            xt_pre[0] = xt0
            wp = []
            for kp in range(NKP):
                w = wp_pool.tile([P, 2, D], F8, tag=f"wp{kp}", bufs=1)
                wp.append(w)
            nc.scalar.dma_start(out=wp[0], in_=w8_d.ap()[0])
            nc.sync.dma_start(out=wp[1], in_=w8_d.ap()[1])
            xt1 = xt_pool.tile([P, NK, P], F8, tag="xt")
            nc.sync.dma_start(out=xt1, in_=xTt_ap[1])
            xt_pre[1] = xt1
            for kp in range(2, NKP):
                eng = nc.scalar if kp % 2 == 0 else nc.sync
                eng.dma_start(out=wp[kp], in_=w8_d.ap()[kp])
            xv_pre = {}
            xv0 = xv_pool.tile([P, D], BF16, tag="xv")
            nc.sync.dma_start(out=xv0[:NR, :], in_=xb_ap[1:1 + NR, :])
            xv_pre[0] = xv0
            nc.scalar.dma_start(out=st_sb, in_=st_d.ap())

            qp_of = {}
            q_of = {}
            qs_of = {}
            v1_of = {}
            v2_of = {}
            rs1_of = {}

            def front(i):
                xt_i = xt_pre.pop(i)
                qp = ps_pool.tile([P, D], FP32, tag="qps", bufs=2)
                for kp in range(NKP):
                    lhsT = xt_i[:, 2 * kp:2 * kp + 2, :]
                    for n in range(NE):
                        nc.tensor.matmul(qp[:, n * 512:(n + 1) * 512],
                                         lhsT,
                                         wp[kp][:, :, n * 512:(n + 1) * 512],
                                         start=(kp == 0), stop=(kp == NKP - 1),
                                         perf_mode=DR)
                qp_of[i] = qp
                if i + 2 < NT:
                    xt_n = xt_pool.tile([P, NK, P], F8, tag="xt")
                    nc.sync.dma_start(out=xt_n, in_=xTt_ap[i + 2])
                    xt_pre[i + 2] = xt_n
                if i + 1 < NT:
                    xv_n = xv_pool.tile([P, D], BF16, tag="xv")
                    t0 = (i + 1) * P + 1
                    nc.sync.dma_start(out=xv_n[:NR, :], in_=xb_ap[t0:t0 + NR, :])
                    xv_pre[i + 1] = xv_n

            def evac(i, chunks=1):
                # PSUM -> SBUF raw copy (bf16) on DVE, one step after front(i)
                q_i = q_pool.tile([P, D], BF16, tag="q")
                qp = qp_of.pop(i)
                cw = D // chunks
                for c in range(chunks):
                    nc.vector.tensor_copy(out=q_i[:, c * cw:(c + 1) * cw],
                                          in_=qp[:, c * cw:(c + 1) * cw])
                q_of[i] = q_i

            def qs_copy(i):
                # shifted-Q operand for the second LN: qs[j] = Q_raw[i*128+2+j]
                # single 126-row HWDGE copy from THIS tile's q only
                qs_i = qs_pool.tile([P, D], BF16, tag="qs")
                nc.sync.dma_start(out=qs_i[0:NR, :], in_=q_of.pop(i)[2:P, :])
                qs_of[i] = qs_i

            sq1_st = {}
            sq2_st = {}

            def add1(i):
                # v1 = x'_t + Q_raw[t-1]  (primed units, plain add)
                xv_i = xv_pre.pop(i)
                q_i = q_of[i]
                v1 = v1_pool.tile([P, D], BF16, tag="v1")
                nc.vector.tensor_add(out=v1[:NR], in0=q_i[:NR],
                                     in1=xv_i[:NR])
                v1_of[i] = v1

            def sq1_start(i):
                # S1 = sum(v1^2) via ACT Square+accum
                sq = sq_pool.tile([P, D], BF16, tag="sq")
                s1a = stat_pool.tile([P, 1], FP32, tag="s1a")
                nc.scalar.activation(out=sq[:NR], in_=v1_of[i][:NR],
                                     func=AF.Square, accum_out=s1a[:NR])
                sq1_st[i] = s1a

            def a_stats(i):
                # rs1 = 1/sigma1 (primed apply keeps LN1 output in primed units)
                s1a = sq1_st.pop(i)
                u1 = stat_pool.tile([P, 1], FP32, tag="u1")
                nc.vector.tensor_add(out=u1[:NR], in0=s1a[:NR], in1=c1(i)[:NR])
                s1 = stat_pool.tile([P, 1], FP32, tag="s1")
                nc.scalar.activation(out=s1[:NR], in_=u1[:NR], func=AF.Sqrt,
                                     scale=qscale * qscale / D)
                rs1 = stat_pool.tile([P, 1], FP32, tag="rs1")
                nc.vector.reciprocal(out=rs1[:NR], in_=s1[:NR])
                rs1_of[i] = rs1

            def apply_b(i):
                # v2 = LN1(v1)/qscale + Q_raw[t+1]
                v1 = v1_of.pop(i)
                rs1 = rs1_of.pop(i)
                v2 = v2_pool.tile([P, D], BF16, tag="v2")
                nc.vector.tensor_scalar(out=v2[:NR], in0=v1[:NR],
                                        scalar1=mu1(i)[:NR], scalar2=rs1[:NR],
                                        op0=ALU.subtract, op1=ALU.mult)
                nc.vector.tensor_add(out=v2[:NR], in0=qs_of.pop(i)[:NR],
                                     in1=v2[:NR])
                v2_of[i] = v2

            def sq2_start(i):
                sqf = sq_pool.tile([P, D], BF16, tag="sq2")
                s2a = stat_pool.tile([P, 1], FP32, tag="s2a")
                nc.scalar.activation(out=sqf[:NR], in_=v2_of[i][:NR],
                                     func=AF.Square, accum_out=s2a[:NR])
                sq2_st[i] = s2a

            def h2_rest(i):
                v2 = v2_of.pop(i)
                s2a = sq2_st.pop(i)
                u2 = stat_pool.tile([P, 1], FP32, tag="u2")
                nc.vector.tensor_add(out=u2[:NR], in0=s2a[:NR], in1=c2(i)[:NR])
                s2 = stat_pool.tile([P, 1], FP32, tag="s2")
                nc.scalar.activation(out=s2[:NR], in_=u2[:NR], func=AF.Sqrt,
                                     scale=qscale * qscale / D)
                rs2 = stat_pool.tile([P, 1], FP32, tag="rs2")
                nc.vector.reciprocal(out=rs2[:NR], in_=s2[:NR])
                # rs2' = qscale/sigma2: converts primed v2 back to unit scale
                rs2p = stat_pool.tile([P, 1], FP32, tag="rs2p")
                nc.vector.tensor_scalar_mul(rs2p[:NR], rs2[:NR], qscale)
                o = o_pool.tile([P, D], BF16, tag="o")
                nc.vector.tensor_scalar(out=o[:NR], in0=v2[:NR],
                                        scalar1=mu2(i)[:NR], scalar2=rs2p[:NR],
                                        op0=ALU.subtract, op1=ALU.mult)
                t0 = i * P + 1
                nc.sync.dma_start(out=out_ap[t0:t0 + NR, :], in_=o[:NR])

            # 4-stage software pipeline with hand-ordered per-engine FIFOs:
            # front(i) | evac+add1+stats(i)@+1 | apply_b(i)@+2 | half2(i)@+3.
            # Emission order is chosen so no engine queue head ever waits
            # long on another engine's in-flight op (the strict FIFOs would
            # otherwise serialize: an op's wait blocks everything behind it).
            def step(i):
                front(i)
                if i >= 1:
                    evac(i - 1)           # DVE: frees PSUM early
                if i >= 3:
                    sq2_start(i - 3)      # ACT: leads the ACT queue, dep ready
                if i >= 1:
                    add1(i - 1)           # DVE
                    sq1_start(i - 1)      # ACT: after Sq2 in queue
                    qs_copy(i - 1)        # sync
                if i >= 2:
                    apply_b(i - 2)        # DVE
                if i >= 3:
                    h2_rest(i - 3)        # DVE smalls + ACT sqrt2 + store
                if i >= 1:
                    a_stats(i - 1)        # DVE smalls + ACT sqrt1

            def front_last(i):
                # n-outer: each 512-col PSUM chunk completes after its 8 kp
                # matmuls, so the drain chain starts ~3/4 of a front early.
                # (LDWEIGHTS becomes rate-limiting, +~1.4us on the stream,
                # but the chain starts ~5.5us earlier: net win on the tail.)
                xt_i = xt_pre.pop(i)
                qp = ps_pool.tile([P, D], FP32, tag="qps", bufs=2)
                for n in range(NE):
                    for kp in range(NKP):
                        lhsT = xt_i[:, 2 * kp:2 * kp + 2, :]
                        nc.tensor.matmul(qp[:, n * 512:(n + 1) * 512],
                                         lhsT,
                                         wp[kp][:, :, n * 512:(n + 1) * 512],
                                         start=(kp == 0), stop=(kp == NKP - 1),
                                         perf_mode=DR)
                qp_of[i] = qp

            def chunk_chain_last(i):
                # evac/add1/Sq1/qs of the last tile, chunked per 512-col
                # PSUM chunk so each starts as soon as its matmuls finish
                q_i = q_pool.tile([P, D], BF16, tag="q")
                qs_i = qs_pool.tile([P, D], BF16, tag="qs")
                v1 = v1_pool.tile([P, D], BF16, tag="v1")
                sq = sq_pool.tile([P, D], BF16, tag="sq")
                xv_i = xv_pre.pop(i)
                qp = qp_of.pop(i)
                accs = []
                for n in range(NE):
                    sl = slice(n * 512, (n + 1) * 512)
                    nc.vector.tensor_copy(out=q_i[:, sl], in_=qp[:, sl])
                    nc.vector.tensor_add(out=v1[:NR, sl], in0=q_i[:NR, sl],
                                         in1=xv_i[:NR, sl])
                    s1n = stat_pool.tile([P, 1], FP32, tag=f"s1c{n}")
                    nc.scalar.activation(out=sq[:NR, sl], in_=v1[:NR, sl],
                                         func=AF.Square, accum_out=s1n[:NR])
                    nc.sync.dma_start(out=qs_i[0:NR, sl], in_=q_i[2:P, sl])
                    accs.append(s1n)
                ua = stat_pool.tile([P, 1], FP32, tag="u1a")
                nc.vector.tensor_add(out=ua[:NR], in0=accs[0][:NR],
                                     in1=accs[1][:NR])
                ub = stat_pool.tile([P, 1], FP32, tag="u1b")
                nc.vector.tensor_add(out=ub[:NR], in0=accs[2][:NR],
                                     in1=accs[3][:NR])
                nc.vector.tensor_add(out=ua[:NR], in0=ua[:NR], in1=ub[:NR])
                u1 = stat_pool.tile([P, 1], FP32, tag="u1")
                nc.vector.tensor_add(out=u1[:NR], in0=ua[:NR], in1=c1(i)[:NR])
                s1 = stat_pool.tile([P, 1], FP32, tag="s1")
                nc.scalar.activation(out=s1[:NR], in_=u1[:NR], func=AF.Sqrt,
                                     scale=qscale * qscale / D)
                rs1 = stat_pool.tile([P, 1], FP32, tag="rs1")
                nc.vector.reciprocal(out=rs1[:NR], in_=s1[:NR])
                v1_of[i] = v1
                rs1_of[i] = rs1
                qs_of[i] = qs_i

            for i in range(NT - 1):
                step(i)
            # last front + drain: tile chains are independent; emission in
            # estimated ready-time order so no engine FIFO head-blocks
            L = NT - 1
            front(L)
            evac(L - 1)
            sq2_start(L - 3)
            add1(L - 1)
            sq1_start(L - 1)
            qs_copy(L - 1)
            apply_b(L - 2)
            h2_rest(L - 3)
            a_stats(L - 1)
            evac(L, chunks=2)
            add1(L)
            sq1_start(L)
            qs_copy(L)
            sq2_start(L - 2)
            apply_b(L - 1)
            h2_rest(L - 2)
            a_stats(L)
            sq2_start(L - 1)
            apply_b(L)
            h2_rest(L - 1)
            sq2_start(L)
            h2_rest(L)

    nc.compile()
    return nc


def _get_program(scale: float):
    key = round(float(scale), 9)
    if key not in _cache:
        _cache[key] = _build(float(scale))
    return _cache[key]


def _identity_ln_params(bp, gamma, beta):
    return (not np.any(bp)) and (not np.any(beta)) and np.all(gamma == 1.0)


def _ln_np(v):
    mu = v.mean(-1, keepdims=True)
    var = ((v - mu) ** 2).mean(-1, keepdims=True)
    return (v - mu) / np.sqrt(var + EPS)


def _reference_numpy(x, W1, b1, W2, b2, Wp, bp, gamma, beta):
    """Exact numpy port of the jax reference (emergency fallback only)."""
    import math

    def ln(v):
        mu = v.mean(-1, keepdims=True)
        var = ((v - mu) ** 2).mean(-1, keepdims=True)
        return (v - mu) / np.sqrt(var + EPS) * gamma + beta

    erf = np.vectorize(math.erf)
    x64 = x.astype(np.float32)
    pair = np.concatenate([x64[:, :-1], x64[:, 1:]], axis=-1)
    h0 = pair @ W1.T + b1
    h = 0.5 * h0 * (1.0 + erf(h0 / np.sqrt(2.0)))
    logits = h @ W2.T + b2
    e = np.exp(logits - logits.max(-1, keepdims=True))
    sm = e / e.sum(-1, keepdims=True)
    scale = sm.mean(-1, keepdims=True)
    Pm = x64 @ Wp.T + bp
    m = Pm[:, 1:] * scale
    mp = Pm[:, :-1] * scale
    tmp = np.concatenate([x64[:, :1], ln(x64[:, 1:] + mp)], axis=1)
    out = np.concatenate([ln(tmp[:, :-1] + m), tmp[:, -1:]], axis=1)
    return out.astype(np.float32)


# device-skipped rows: t=0, T-1 and the 2 rows at each 126-row tile boundary
HOST_TS = sorted({0, T - 1} |
                 {i * P + 127 for i in range(NT - 1)} |
                 {i * P + 128 for i in range(NT - 1)})
_P_ROWS = sorted({1, T - 2} |
                 {i * P + o for i in range(NT - 1) for o in (126, 127, 128, 129)})
_P_IDX = {r: k for k, r in enumerate(_P_ROWS)}


def _stats_table(x_c, rowsum, scale, qscale):
    """Host LN stats in PRIMED units (v' = v/qscale): [P, 4*NT] fp32."""
    xsum = x_c.sum(-1, dtype=np.float64)
    Qsum = (x_c @ rowsum).astype(np.float64)
    mu1 = np.zeros(T)
    mu1[1:] = (xsum[1:] + scale * Qsum[:-1]) / D
    mu2 = np.zeros(T)
    mu2[:T - 1] = scale * Qsum[1:] / D
    t_idx = np.arange(NT)[None, :] * P + 1 + np.arange(P)[:, None]  # [P, NT]
    ok = np.arange(P)[:, None] < NR
    ti = np.minimum(t_idx, T - 1)
    m1 = np.where(ok, mu1[ti], 0.0)
    m2 = np.where(ok, mu2[ti], 0.0)
    cc1 = (EPS - m1 ** 2) * D
    cc2 = (EPS - m2 ** 2) * D
    q2 = qscale * qscale
    return np.ascontiguousarray(
        np.concatenate([m1 / qscale, cc1 / q2, m2 / qscale, cc2 / q2],
                       axis=1).astype(np.float32))


def _host_rows(x, wT, scale, out):
    """Fill the device-skipped rows exactly on host (one batched GEMM)."""
    Pn = x[:, _P_ROWS, :].astype(np.float64) @ wT.astype(np.float64)  # (B,R,D)
    for c in range(N_CORES):
        for t in HOST_TS:
            if t == 0:
                tmp = x[c, 0].astype(np.float64)
            else:
                tmp = _ln_np(x[c, t].astype(np.float64)
                             + scale * Pn[c, _P_IDX[t - 1]])
            if t == T - 1:
                out[c, t] = tmp
            else:
                out[c, t] = _ln_np(tmp + scale * Pn[c, _P_IDX[t + 1]])


def run_device(x, wT, scale, trace=False):
    """x: (B,T,D) fp32, wT: (D,D) fp32 (= Wp.T contiguous)."""
    nc = _get_program(scale)
    x8 = np.clip(x * SX, -240.0, 240.0).astype(F8NP)         # (B,T,D) fp8
    w8 = np.ascontiguousarray(
        np.clip(wT * SW, -240.0, 240.0).astype(F8NP)
        .reshape(NKP, 2, P, D).transpose(0, 2, 1, 3))        # (8,128,2,2048)
    rowsum = wT.sum(1).astype(np.float32)
    qscale = float(scale) / (SX * SW)
    in_maps = []
    for c in range(N_CORES):
        # x in primed units (x/qscale) so the device adds need no scaling
        xb = np.ascontiguousarray((x[c] * (1.0 / qscale)).astype(BF16NP))
        # xTt[i, p, k, tt] = x8[i*128+tt, k*128+p]
        xTb = np.ascontiguousarray(
            x8[c].reshape(NT, P, NK, P).transpose(0, 3, 2, 1))
        st = _stats_table(x[c], rowsum, scale, qscale)
        in_maps.append({"xb": xb, "xTt": xTb, "w8": w8, "st": st})
    res = run_bass_kernel_spmd(nc, in_maps, list(range(N_CORES)), trace=trace)
    out = np.empty((B, T, D), np.float32)
    for c in range(N_CORES):
        out[c] = res.results[c]["out"].astype(np.float32)
    _host_rows(x, wT, scale, out)
    return out, res


def kernel(x, W1, b1, W2, b2, Wp, bp, gamma, beta):
    x = np.asarray(x, dtype=np.float32)
    Wp = np.asarray(Wp, dtype=np.float32)
    bp = np.asarray(bp); gamma = np.asarray(gamma); beta = np.asarray(beta)
    b2 = np.asarray(b2)
    if x.shape != (B, T, D) or not _identity_ln_params(bp, gamma, beta):
        return _reference_numpy(np.asarray(x), np.asarray(W1), np.asarray(b1),
                                np.asarray(W2), b2, Wp, bp, gamma, beta)
    scale = 1.0 / float(b2.shape[0])
    wT = np.ascontiguousarray(Wp.T)
    out, _ = run_device(x, wT, scale, trace=False)
    return out


# revision 29
# speedup vs baseline: 1.0592x; 1.0579x over previous
"""Trainium2 Bass kernel for nn_BraidCrossing (B=8, T=2048, D=2048, NG=3).

Math notes
----------
reference computes:
    pair  = [x_t, x_{t+1}]                       (B, T-1, 2D)
    h     = gelu(pair @ W1.T + b1)
    logit = h @ W2.T + b2                        (B, T-1, 2*NG)
    scale = mean(softmax(logit, -1), -1)         == 1/(2*NG) EXACTLY (mean of a
                                                 softmax over the same axis)
    P     = x @ Wp.T + bp
    tmp_t = LN(x_t + P_{t-1} * scale)   t>=1 ;  tmp_0 = x_0
    out_t = LN(tmp_t + P_{t+1} * scale) t<=T-2; out_{T-1} = tmp_{T-1}

Because scale is a constant (1/(2*NG); setup has bp=0, gamma=1, beta=0), the
entire W1/W2/gelu branch is dead code.  The device kernel computes
Q = x @ Wp.T, then the two chained layernorms (scale folded into the adds).

Key structural tricks:
 * LN *means* are linear in x -> computed on the HOST exactly:
   mu1[t] = (sum_e x[t] + scale * Qsum[t-1]) / D, Qsum[t] = x[t] . rowsum(WpT),
   and mean(LN1_out) == 0 by construction so mu2[t] = scale * Qsum[t+1] / D.
   Only the variances need a device-side quadratic reduction, done with
   ACT activation(Square, accum_out), one pass per LN.  (NOTE:
   vector.tensor_tensor_reduce crashes the exec unit on this runtime —
   NRT_EXEC_UNIT_UNRECOVERABLE — do not use it.)
 * UNIFORM 126-row tiles: tile i handles out rows t = i*128+1 .. i*128+126.
   The 2 rows at each tile boundary (t = i*128+127, i*128+128), plus t=0 and
   t=T-1, are single matvecs -> computed on host (32 rows/core, one small
   batched GEMM).  This removes ALL cross-tile dependencies (the shifted-Q
   operand q[t+1] comes from rows 2..127 of the SAME tile's Q) and all tiny
   2-row DMAs (whose completion semaphores measured ~8-10us latency when the
   HWDGE ring idles — that stall serialized the old drain).
 * PSUM evacuation of tile i happens one pipeline step AFTER its matmuls,
   as a raw tensor_copy (scale folded into the downstream STT adds), so no
   engine dependency ever gates the PE matmul stream (HAM stays warm).
 * Output stored bf16, upcast on host (halves store traffic).
 * Matmul loop kp-outer/n-inner: 4 consecutive MMs share one LDWEIGHTS
   content; n-outer ordering makes LDWEIGHTS rate-limiting (216->259ns/MM).

Precision: GEMM in fp8 e4m3 (DoubleRow, K=256/matmul, fp32 PSUM); LN chain in
bf16 with fp32 statistics.  Measured max rel err ~1.3e-2 (gate 2e-2).

Sharding: data-parallel over batch, one batch per NeuronCore (8 cores).
"""
import numpy as np
import ml_dtypes

import concourse.bass as bass
from concourse import bacc
import concourse.mybir as mybir
import concourse.tile as tile
from concourse.bass_utils import run_bass_kernel_spmd

FP32 = mybir.dt.float32
BF16 = mybir.dt.bfloat16
F8 = mybir.dt.float8e4
AF = mybir.ActivationFunctionType
ALU = mybir.AluOpType
DR = mybir.MatmulPerfMode.DoubleRow

B, T, D = 8, 2048, 2048
P = 128                # partitions
NR = P - 2             # 126 device rows per tile
NT = T // P            # 16 t-tiles
NK = D // P            # 16 contraction k-tiles
NKP = NK // 2          # 8 k-pairs (DoubleRow: 256 contraction per matmul)
NE = D // 512          # 4 psum-bank chunks along e
EPS = 1e-5
N_CORES = 8

SX = 16.0              # fp8 pre-scale for x
SW = 1024.0            # fp8 pre-scale for Wp.T
F8NP = ml_dtypes.float8_e4m3
BF16NP = ml_dtypes.bfloat16

_cache = {}


def _build(scale: float):
    # PSUM = (x*SX) @ (WpT*SW); q_raw = copy(PSUM);
    # q = q_raw * qscale applied inside the two STT adds
    qscale = float(scale) / (SX * SW)

    nc = bacc.Bacc("TRN2", target_bir_lowering=False, debug=False)
    xb_d = nc.declare_dram_parameter("xb", [T, D], BF16, isOutput=False)
    # host-tiled transpose: xTt[i, p, k, tt] = x[i*128+tt, k*128+p] (fp8),
    # so lhsT slice [:, 2kp:2kp+2, :] is the DoubleRow stationary operand
    xTt_d = nc.declare_dram_parameter("xTt", [NT, P, NK, P], F8, isOutput=False)
    # w8[kp, p, s, e] = WpT[(2kp+s)*128+p, e] * SW (fp8)
    w8_d = nc.declare_dram_parameter("w8", [NKP, P, 2, D], F8, isOutput=False)
    # host LN stats: columns [mu1 | c1 | mu2 | c2], c = (eps - mu^2) * D
    st_d = nc.declare_dram_parameter("st", [P, 4 * NT], FP32, isOutput=False)
    out_d = nc.declare_dram_parameter("out", [T, D], BF16, isOutput=True)

    xb_ap = xb_d.ap()
    out_ap = out_d.ap()
    xTt_ap = xTt_d.ap()

    with tile.TileContext(nc) as tc:
        with tc.tile_pool(name="wp", bufs=1) as wp_pool, \
             tc.tile_pool(name="xt", bufs=3) as xt_pool, \
             tc.tile_pool(name="q", bufs=3) as q_pool, \
             tc.tile_pool(name="xv", bufs=4) as xv_pool, \
             tc.tile_pool(name="qs", bufs=3) as qs_pool, \
             tc.tile_pool(name="v1", bufs=3) as v1_pool, \
             tc.tile_pool(name="sq", bufs=2) as sq_pool, \
             tc.tile_pool(name="v2", bufs=3) as v2_pool, \
             tc.tile_pool(name="o", bufs=3) as o_pool, \
             tc.tile_pool(name="stat", bufs=4) as stat_pool, \
             tc.tile_pool(name="ps", bufs=2, space="PSUM") as ps_pool:

            st_sb = stat_pool.tile([P, 4 * NT], FP32, tag="st", bufs=1)

            def mu1(i):
                return st_sb[:, i:i + 1]

            def c1(i):
                return st_sb[:, NT + i:NT + i + 1]

            def mu2(i):
                return st_sb[:, 2 * NT + i:2 * NT + i + 1]

            def c2(i):
                return st_sb[:, 3 * NT + i:3 * NT + i + 1]

            # prefetch first lhsT tile and x rows, then stream the fp8
            # weights kp-ordered across both HWDGE rings: front(0)'s kp=0
            # matmuls start as soon as xt(0) and wp[0] land
            xt_pre = {}
            xt0 = xt_pool.tile([P, NK, P], F8, tag="xt")
            nc.sync.dma_start(out=xt0, in_=xTt_ap[0])
            xt_pre[0] = xt0
            wp = []
            for kp in range(NKP):
                w = wp_pool.tile([P, 2, D], F8, tag=f"wp{kp}", bufs=1)
                eng = nc.scalar if kp % 2 == 0 else nc.sync
                eng.dma_start(out=w, in_=w8_d.ap()[kp])
                wp.append(w)
            xt1 = xt_pool.tile([P, NK, P], F8, tag="xt")
            nc.sync.dma_start(out=xt1, in_=xTt_ap[1])
            xt_pre[1] = xt1
            xv_pre = {}
            xv0 = xv_pool.tile([P, D], BF16, tag="xv")
            nc.sync.dma_start(out=xv0[:NR, :], in_=xb_ap[1:1 + NR, :])
            xv_pre[0] = xv0
            nc.scalar.dma_start(out=st_sb, in_=st_d.ap())

            qp_of = {}
            q_of = {}
            qs_of = {}
            v1_of = {}
            v2_of = {}
            rs1_of = {}

            def front(i):
                xt_i = xt_pre.pop(i)
                qp = ps_pool.tile([P, D], FP32, tag="qps", bufs=2)
                for kp in range(NKP):
                    lhsT = xt_i[:, 2 * kp:2 * kp + 2, :]
                    for n in range(NE):
                        nc.tensor.matmul(qp[:, n * 512:(n + 1) * 512],
                                         lhsT,
                                         wp[kp][:, :, n * 512:(n + 1) * 512],
                                         start=(kp == 0), stop=(kp == NKP - 1),
                                         perf_mode=DR)
                qp_of[i] = qp
                if i + 2 < NT:
                    xt_n = xt_pool.tile([P, NK, P], F8, tag="xt")
                    nc.sync.dma_start(out=xt_n, in_=xTt_ap[i + 2])
                    xt_pre[i + 2] = xt_n
                if i + 1 < NT:
                    xv_n = xv_pool.tile([P, D], BF16, tag="xv")
                    t0 = (i + 1) * P + 1
                    nc.sync.dma_start(out=xv_n[:NR, :], in_=xb_ap[t0:t0 + NR, :])
                    xv_pre[i + 1] = xv_n

            def evac(i, chunks=1):
                # PSUM -> SBUF raw copy (bf16) on DVE, one step after front(i)
                q_i = q_pool.tile([P, D], BF16, tag="q")
                qp = qp_of.pop(i)
                cw = D // chunks
                for c in range(chunks):
                    nc.vector.tensor_copy(out=q_i[:, c * cw:(c + 1) * cw],
                                          in_=qp[:, c * cw:(c + 1) * cw])
                q_of[i] = q_i

            def qs_copy(i):
                # shifted-Q operand for the second LN: qs[j] = Q_raw[i*128+2+j]
                # single 126-row HWDGE copy from THIS tile's q only
                qs_i = qs_pool.tile([P, D], BF16, tag="qs")
                nc.sync.dma_start(out=qs_i[0:NR, :], in_=q_of.pop(i)[2:P, :])
                qs_of[i] = qs_i

            sq1_st = {}
            sq2_st = {}

            def add1(i):
                # v1 = x'_t + Q_raw[t-1]  (primed units, plain add)
                xv_i = xv_pre.pop(i)
                q_i = q_of[i]
                v1 = v1_pool.tile([P, D], BF16, tag="v1")
                nc.vector.tensor_add(out=v1[:NR], in0=q_i[:NR],
                                     in1=xv_i[:NR])
                v1_of[i] = v1

            def sq1_start(i):
                # S1 = sum(v1^2) via ACT Square+accum
                sq = sq_pool.tile([P, D], BF16, tag="sq")
                s1a = stat_pool.tile([P, 1], FP32, tag="s1a")
                nc.scalar.activation(out=sq[:NR], in_=v1_of[i][:NR],
                                     func=AF.Square, accum_out=s1a[:NR])
                sq1_st[i] = s1a

            def a_stats(i):
                # rs1 = 1/sigma1 (primed apply keeps LN1 output in primed units)
                s1a = sq1_st.pop(i)
                u1 = stat_pool.tile([P, 1], FP32, tag="u1")
                nc.vector.tensor_add(out=u1[:NR], in0=s1a[:NR], in1=c1(i)[:NR])
                s1 = stat_pool.tile([P, 1], FP32, tag="s1")
                nc.scalar.activation(out=s1[:NR], in_=u1[:NR], func=AF.Sqrt,
                                     scale=qscale * qscale / D)
                rs1 = stat_pool.tile([P, 1], FP32, tag="rs1")
                nc.vector.reciprocal(out=rs1[:NR], in_=s1[:NR])
                rs1_of[i] = rs1

            def apply_b(i):
                # v2 = LN1(v1)/qscale + Q_raw[t+1]
                v1 = v1_of.pop(i)
                rs1 = rs1_of.pop(i)
                v2 = v2_pool.tile([P, D], BF16, tag="v2")
                nc.vector.tensor_scalar(out=v2[:NR], in0=v1[:NR],
                                        scalar1=mu1(i)[:NR], scalar2=rs1[:NR],
                                        op0=ALU.subtract, op1=ALU.mult)
                nc.vector.tensor_add(out=v2[:NR], in0=qs_of.pop(i)[:NR],
                                     in1=v2[:NR])
                v2_of[i] = v2

            def sq2_start(i):
                sqf = sq_pool.tile([P, D], BF16, tag="sq2")
                s2a = stat_pool.tile([P, 1], FP32, tag="s2a")
                nc.scalar.activation(out=sqf[:NR], in_=v2_of[i][:NR],
                                     func=AF.Square, accum_out=s2a[:NR])
                sq2_st[i] = s2a

            def h2_rest(i):
                v2 = v2_of.pop(i)
                s2a = sq2_st.pop(i)
                u2 = stat_pool.tile([P, 1], FP32, tag="u2")
                nc.vector.tensor_add(out=u2[:NR], in0=s2a[:NR], in1=c2(i)[:NR])
                s2 = stat_pool.tile([P, 1], FP32, tag="s2")
                nc.scalar.activation(out=s2[:NR], in_=u2[:NR], func=AF.Sqrt,
                                     scale=qscale * qscale / D)
                rs2 = stat_pool.tile([P, 1], FP32, tag="rs2")
                nc.vector.reciprocal(out=rs2[:NR], in_=s2[:NR])
                # rs2' = qscale/sigma2: converts primed v2 back to unit scale
                rs2p = stat_pool.tile([P, 1], FP32, tag="rs2p")
                nc.vector.tensor_scalar_mul(rs2p[:NR], rs2[:NR], qscale)
                o = o_pool.tile([P, D], BF16, tag="o")
                nc.vector.tensor_scalar(out=o[:NR], in0=v2[:NR],
                                        scalar1=mu2(i)[:NR], scalar2=rs2p[:NR],
                                        op0=ALU.subtract, op1=ALU.mult)
                t0 = i * P + 1
                nc.sync.dma_start(out=out_ap[t0:t0 + NR, :], in_=o[:NR])

            # 4-stage software pipeline with hand-ordered per-engine FIFOs:
            # front(i) | evac+add1+stats(i)@+1 | apply_b(i)@+2 | half2(i)@+3.
            # Emission order is chosen so no engine queue head ever waits
            # long on another engine's in-flight op (the strict FIFOs would
            # otherwise serialize: an op's wait blocks everything behind it).
            def step(i):
                front(i)
                if i >= 1:
                    evac(i - 1)           # DVE: frees PSUM early
                if i >= 3:
                    sq2_start(i - 3)      # ACT: leads the ACT queue, dep ready
                if i >= 1:
                    add1(i - 1)           # DVE
                    sq1_start(i - 1)      # ACT: after Sq2 in queue
                    qs_copy(i - 1)        # sync
                if i >= 2:
                    apply_b(i - 2)        # DVE
                if i >= 3:
                    h2_rest(i - 3)        # DVE smalls + ACT sqrt2 + store
                if i >= 1:
                    a_stats(i - 1)        # DVE smalls + ACT sqrt1

            for i in range(NT - 1):
                step(i)
            # last front + drain: tile chains are independent; emission in
            # estimated ready-time order so no engine FIFO head-blocks
            L = NT - 1
            front(L)
            evac(L - 1)
            sq2_start(L - 3)
            add1(L - 1)
            sq1_start(L - 1)
            qs_copy(L - 1)
            apply_b(L - 2)
            h2_rest(L - 3)
            a_stats(L - 1)
            evac(L, chunks=2)
            add1(L)
            sq1_start(L)
            qs_copy(L)
            sq2_start(L - 2)
            apply_b(L - 1)
            h2_rest(L - 2)
            a_stats(L)
            sq2_start(L - 1)
            apply_b(L)
            h2_rest(L - 1)
            sq2_start(L)
            h2_rest(L)

    nc.compile()
    return nc


def _get_program(scale: float):
    key = round(float(scale), 9)
    if key not in _cache:
        _cache[key] = _build(float(scale))
    return _cache[key]


def _identity_ln_params(bp, gamma, beta):
    return (not np.any(bp)) and (not np.any(beta)) and np.all(gamma == 1.0)


def _ln_np(v):
    mu = v.mean(-1, keepdims=True)
    var = ((v - mu) ** 2).mean(-1, keepdims=True)
    return (v - mu) / np.sqrt(var + EPS)


def _reference_numpy(x, W1, b1, W2, b2, Wp, bp, gamma, beta):
    """Exact numpy port of the jax reference (emergency fallback only)."""
    import math

    def ln(v):
        mu = v.mean(-1, keepdims=True)
        var = ((v - mu) ** 2).mean(-1, keepdims=True)
        return (v - mu) / np.sqrt(var + EPS) * gamma + beta

    erf = np.vectorize(math.erf)
    x64 = x.astype(np.float32)
    pair = np.concatenate([x64[:, :-1], x64[:, 1:]], axis=-1)
    h0 = pair @ W1.T + b1
    h = 0.5 * h0 * (1.0 + erf(h0 / np.sqrt(2.0)))
    logits = h @ W2.T + b2
    e = np.exp(logits - logits.max(-1, keepdims=True))
    sm = e / e.sum(-1, keepdims=True)
    scale = sm.mean(-1, keepdims=True)
    Pm = x64 @ Wp.T + bp
    m = Pm[:, 1:] * scale
    mp = Pm[:, :-1] * scale
    tmp = np.concatenate([x64[:, :1], ln(x64[:, 1:] + mp)], axis=1)
    out = np.concatenate([ln(tmp[:, :-1] + m), tmp[:, -1:]], axis=1)
    return out.astype(np.float32)


# device-skipped rows: t=0, T-1 and the 2 rows at each 126-row tile boundary
HOST_TS = sorted({0, T - 1} |
                 {i * P + 127 for i in range(NT - 1)} |
                 {i * P + 128 for i in range(NT - 1)})
_P_ROWS = sorted({1, T - 2} |
                 {i * P + o for i in range(NT - 1) for o in (126, 127, 128, 129)})
_P_IDX = {r: k for k, r in enumerate(_P_ROWS)}


def _stats_table(x_c, rowsum, scale, qscale):
    """Host LN stats in PRIMED units (v' = v/qscale): [P, 4*NT] fp32."""
    xsum = x_c.sum(-1, dtype=np.float64)
    Qsum = (x_c @ rowsum).astype(np.float64)
    mu1 = np.zeros(T)
    mu1[1:] = (xsum[1:] + scale * Qsum[:-1]) / D
    mu2 = np.zeros(T)
    mu2[:T - 1] = scale * Qsum[1:] / D
    t_idx = np.arange(NT)[None, :] * P + 1 + np.arange(P)[:, None]  # [P, NT]
    ok = np.arange(P)[:, None] < NR
    ti = np.minimum(t_idx, T - 1)
    m1 = np.where(ok, mu1[ti], 0.0)
    m2 = np.where(ok, mu2[ti], 0.0)
    cc1 = (EPS - m1 ** 2) * D
    cc2 = (EPS - m2 ** 2) * D
    q2 = qscale * qscale
    return np.ascontiguousarray(
        np.concatenate([m1 / qscale, cc1 / q2, m2 / qscale, cc2 / q2],
                       axis=1).astype(np.float32))


def _host_rows(x, wT, scale, out):
    """Fill the device-skipped rows exactly on host (one batched GEMM)."""
    Pn = x[:, _P_ROWS, :].astype(np.float64) @ wT.astype(np.float64)  # (B,R,D)
    for c in range(N_CORES):
        for t in HOST_TS:
            if t == 0:
                tmp = x[c, 0].astype(np.float64)
            else:
                tmp = _ln_np(x[c, t].astype(np.float64)
                             + scale * Pn[c, _P_IDX[t - 1]])
            if t == T - 1:
                out[c, t] = tmp
            else:
                out[c, t] = _ln_np(tmp + scale * Pn[c, _P_IDX[t + 1]])


def run_device(x, wT, scale, trace=False):
    """x: (B,T,D) fp32, wT: (D,D) fp32 (= Wp.T contiguous)."""
    nc = _get_program(scale)
    x8 = np.clip(x * SX, -240.0, 240.0).astype(F8NP)         # (B,T,D) fp8
    w8 = np.ascontiguousarray(
        np.clip(wT * SW, -240.0, 240.0).astype(F8NP)
        .reshape(NKP, 2, P, D).transpose(0, 2, 1, 3))        # (8,128,2,2048)
    rowsum = wT.sum(1).astype(np.float32)
    qscale = float(scale) / (SX * SW)
    in_maps = []
    for c in range(N_CORES):
        # x in primed units (x/qscale) so the device adds need no scaling
        xb = np.ascontiguousarray((x[c] * (1.0 / qscale)).astype(BF16NP))
        # xTt[i, p, k, tt] = x8[i*128+tt, k*128+p]
        xTb = np.ascontiguousarray(
            x8[c].reshape(NT, P, NK, P).transpose(0, 3, 2, 1))
        st = _stats_table(x[c], rowsum, scale, qscale)
        in_maps.append({"xb": xb, "xTt": xTb, "w8": w8, "st": st})
    res = run_bass_kernel_spmd(nc, in_maps, list(range(N_CORES)), trace=trace)
    out = np.empty((B, T, D), np.float32)
    for c in range(N_CORES):
        out[c] = res.results[c]["out"].astype(np.float32)
    _host_rows(x, wT, scale, out)
    return out, res


def kernel(x, W1, b1, W2, b2, Wp, bp, gamma, beta):
    x = np.asarray(x, dtype=np.float32)
    Wp = np.asarray(Wp, dtype=np.float32)
    bp = np.asarray(bp); gamma = np.asarray(gamma); beta = np.asarray(beta)
    b2 = np.asarray(b2)
    if x.shape != (B, T, D) or not _identity_ln_params(bp, gamma, beta):
        return _reference_numpy(np.asarray(x), np.asarray(W1), np.asarray(b1),
                                np.asarray(W2), b2, Wp, bp, gamma, beta)
    scale = 1.0 / float(b2.shape[0])
    wT = np.ascontiguousarray(Wp.T)
    out, _ = run_device(x, wT, scale, trace=False)
    return out
